# revision 7
# baseline (speedup 1.0000x reference)
"""
MiniBatchDiscrimination on 8 Trainium2 NeuronCores (Bass/Tile, SPMD).

Reference computation (jax):
    M = (x @ T.reshape(1024, 2048)).reshape(512, 64, 32)
    abs_diff[i, j, o] = sum_k |M[j, o, k] - M[i, o, k]|        # [512, 512, 64]
    feats[i, o]      = sum_j exp(-abs_diff[i, j, o])           # [512, 64]
    out = concat([x, feats], axis=1)                           # [512, 1088]

Distribution strategy (SPMD: one program on 8 cores; all per-core variation
rides in the input data): every core receives x^T ROLLED by -64*core rows
plus the full (replicated) T, computes the full M^T = (x @ T)^T locally, and
produces features for its LOCAL rows 0..63.

Symmetric halving via a cyclic block-window: with 64 blocks of BLK=8 rows,
the row-pass of row i covers columns [8*(i//8), +264) -- its own block plus
the next 32 blocks (no wrap occurs locally since local rows live in blocks
0..7).  For block-distance 1..31 pairs the transpose term is supplied by a
column-accumulator over the window's blocks +1..+31; block-distance-32 pairs
are computed by BOTH owning rows' passes (and excluded from the col-acc), so
every unordered pair contributes to both features exactly once.  This is
0.52x the full pairwise work.  The per-core roll keeps it SPMD-exact: the
scheme only references LOCAL block structure, and the host re-rolls the
column accumulator when folding.

M^T uses a K-MAJOR column order (flat index = k*64 + o) so every one of the
16 partition-chunks maps to output features with the SAME [128, 64] 0/1
stationary; row i0 of a pair reduces into PSUM partitions 0..63 and row i1
into 64..127 (PE tile positioning), sharing one PSUM tile.  T is PRE-SCALED
by 0.5 on the host (exact in bf16); the exp uses scale=-2 to compensate
(harmless numerically, keeps all dtypes comfortably in range).

Device pipeline per core:
  1. PE warm-up burst (~10 dummy matmuls) releases the HAM clock throttle
     before the real GEMM arrives; DMA x^T (1MB) interleaved with T's first
     output-chunk block, then the rest of T okc-major (4MB total).
  2. PE GEMM  M^T = T^T @ x^T (bf16 in, fp32 accum) per okc chunk, evicted
     by ScalarE to bf16 MT [128, 16, 512] plus an fp32 upcast MTf (bias/
     scalar operands must be fp32 AND bit-exact with the bf16 values so
     self-distances are exactly 0).  The first two chunks are emitted ahead,
     then each remaining chunk is emitted interleaved with group 0's
     pairwise work so every consumer waits only on its own chunk.
  3. Per group of GRP=4 row-pairs (one 8-row block), chunk-major:
       - |M^T - m_i| over the 264-wide window into a bf16 tile [128, 8, 264]:
         ScalarE activation(Abs, scale=-1, bias=m_i) for some rows, DVE
         tensor_scalar(subtract) + one batched u16-view bitwise-AND 0x7FFF
         (sign-bit clear = abs) for the rest.
       - k-reduction on PE: per chunk one matmul per row with the shared
         [128, 64] 0/1 stationary slab, accumulating D [128, 264] in PSUM.
       - ScalarE activation(Exp, scale=-2, accum_out) fuses exp(-2D) and
         the window row-sum -> R[:, l]; GpSimdE adds E's blocks +1..+31 into
         the column accumulator ACC [128, 320].
  4. DMA R [128, 32] and ACC [128, 320] back; host scatters/folds.

bf16 in the pairwise stage is safe here: pairwise L1 half-distances of this
input distribution are ~250-700 (exp underflows to exactly 0 in fp32, as in
the reference itself -- min off-diagonal distance measured 502), and
self-terms are exactly 0 in any precision.
"""

import os
import sys

import numpy as np

for _p in ("/opt/trn_rl_repo", "/root/.axon_site/_ro/trn_rl_repo"):
    if os.path.isdir(_p) and _p not in sys.path:
        sys.path.insert(0, _p)

B = 512          # batch
IN_F = 1024      # in_features
OUT_F = 64       # out_features
K = 32           # intermediate dim
OK = OUT_F * K   # 2048 flattened (k, o) -- k-major
P = 128          # partitions
NCHUNK = OK // P      # 16
NCC = IN_F // P       # 8 contraction chunks for the GEMM
NCORES = 8
RPC = B // NCORES     # rows per core = 64
NPAIR = RPC // 2      # 32 row-pairs per core
BLK = int(os.environ.get("MBD_BLK", "8"))   # window block size
WIN = BLK + 256       # own block + 256/BLK more blocks
CA_LO, CA_HI = BLK, 256  # window-relative col-acc range (blocks +1..+m-1)
ACC_W = 320           # max jstart (56) + WIN (264)

# abs-diff engine split: chunks in ACT_CHUNKS run fully on ScalarE, the
# first non-ACT chunk donates SPLIT_ROWS rows to ScalarE, rest on DVE
ACT_CHUNKS = tuple(
    int(c) for c in os.environ.get("MBD_ACT", "2,5,8,11,14").split(",") if c != ""
)
A_BUFS = int(os.environ.get("MBD_ABUFS", "16"))
GRP = int(os.environ.get("MBD_GRP", "4"))  # row-pairs per PSUM group
SPLIT_ROWS = int(os.environ.get("MBD_SPLIT", "4"))  # rows of one DVE chunk -> ACT
N_WARM = int(os.environ.get("MBD_WARM", "10"))  # PE warm-up matmuls

_CACHE = {}


def _stationary():
    """[128, 2, 128] 0/1 matrices: partition (k2, o64) -> PSUM row (k-major).
    Slab 0 maps to rows o (pair row i0), slab 1 to rows 64+o (row i1)."""
    s = np.zeros((P, 2, P), np.float32)
    for p in range(P):
        s[p, 0, p % OUT_F] = 1.0
        s[p, 1, OUT_F + p % OUT_F] = 1.0
    return s


def _build_kernel(tc, r_out, acc_out, x_in, t_in, s_in):
    import concourse.bass as bass
    from concourse import mybir

    nc = tc.nc
    f32 = mybir.dt.float32
    bf16 = mybir.dt.bfloat16
    u16 = mybir.dt.uint16
    SUB = mybir.AluOpType.subtract
    AND = mybir.AluOpType.bitwise_and
    ABS = mybir.ActivationFunctionType.Abs
    EXP = mybir.ActivationFunctionType.Exp

    from contextlib import ExitStack

    with ExitStack() as ctx:
        const = ctx.enter_context(tc.tile_pool(name="const", bufs=1))
        big = ctx.enter_context(tc.tile_pool(name="big", bufs=1))
        staging = ctx.enter_context(tc.tile_pool(name="staging", bufs=1))
        psum_g = ctx.enter_context(tc.tile_pool(name="psum_g", bufs=2, space="PSUM"))

        MT = big.tile([P, NCHUNK, B], bf16)             # 2MB
        MTf = big.tile([P, NCHUNK, B], f32)             # 4MB
        Sb = const.tile([P, 2, P], bf16)
        Rt = const.tile([P, NPAIR], f32)
        ACC = const.tile([P, ACC_W], f32)
        Wz = const.tile([P, B], bf16)
        nc.vector.memset(ACC[:], 0.0)
        nc.vector.memset(Wz[:], 0.0)

        # ---- PE warm-up: release the HAM clock throttle while DMAs fly ----
        pw = psum_g.tile([P, B], f32, tag="pg", name="warm")
        for _ in range(N_WARM):
            nc.tensor.matmul(pw[:], Wz[:, 0:P], Wz[:], start=True, stop=True,
                             skip_group_check=True)

        # ---- input DMAs: x^T interleaved with T's okc-0 block, rest of T
        #      okc-major so GEMM chunk okc unblocks in order ----
        XTb = staging.tile([P, NCC, B], bf16)           # 1MB
        Tb = staging.tile([P, NCHUNK, NCC, P], bf16)    # 4MB
        for cc in range(NCC):
            nc.sync.dma_start(out=XTb[:, cc, :], in_=x_in[cc * P:(cc + 1) * P, :])
            nc.sync.dma_start(out=Tb[:, 0, cc, :], in_=t_in[0, cc, :, :])
        for okc in range(1, NCHUNK):
            for cc in range(NCC):
                nc.sync.dma_start(out=Tb[:, okc, cc, :], in_=t_in[okc, cc, :, :])
        Sf = staging.tile([P, 2, P], f32)
        nc.sync.dma_start(out=Sf[:], in_=s_in[:])
        nc.vector.tensor_copy(out=Sb[:], in_=Sf[:])

        def emit_gemm_chunk(okc):
            pg = psum_g.tile([P, B], f32, tag="pg", name=f"pg{okc}")
            for cc in range(NCC):
                nc.tensor.matmul(
                    pg[:],
                    Tb[:, okc, cc, :],
                    XTb[:, cc, :],
                    start=(cc == 0),
                    stop=(cc == NCC - 1),
                )
            nc.scalar.copy(out=MT[:, okc, :], in_=pg[:])
            nc.scalar.copy(out=MTf[:, okc, :], in_=MT[:, okc, :])

        # ---- pairwise stage ----
        apool = ctx.enter_context(tc.tile_pool(name="apool", bufs=A_BUFS))
        epool = ctx.enter_context(tc.tile_pool(name="epool", bufs=6))
        psum_d = ctx.enter_context(tc.tile_pool(name="psum_d", bufs=6, space="PSUM"))
        act_chunks = set(ACT_CHUNKS)
        split_chunk = next(c for c in range(NCHUNK) if c not in act_chunks)

        NR = 2 * GRP  # rows per group
        dt_tiles = {}

        def emit_pairwise_chunk(g, c):
            pairs = range(g * GRP, (g + 1) * GRP)
            r0 = 2 * g * GRP
            gjs = BLK * (r0 // BLK)
            A8 = apool.tile([P, NR, WIN], bf16, tag="A8", name=f"A8_{c}_{g}")
            n_act = (NR if c in act_chunks
                     else (SPLIT_ROWS if c == split_chunk else 0))
            nd = NR - n_act
            for r in range(NR):
                row = r0 + r
                if r >= nd:
                    nc.scalar.activation(
                        out=A8[:, r, :], in_=MT[:, c, gjs:gjs + WIN], func=ABS,
                        bias=MTf[:, c, row:row + 1], scale=-1.0,
                    )
                else:
                    nc.vector.tensor_scalar(
                        out=A8[:, r, :],
                        in0=MT[:, c, gjs:gjs + WIN],
                        scalar1=MTf[:, c, row:row + 1],
                        scalar2=None, op0=SUB,
                    )
            if nd:
                # batched sign-bit clear (= abs) over the DVE-written rows
                Au = A8[:, 0:nd, :].bitcast(u16)
                nc.vector.tensor_scalar(
                    out=Au, in0=Au, scalar1=0x7FFF, scalar2=None, op0=AND,
                )
            for l in pairs:
                lr = 2 * (l - g * GRP)
                nc.tensor.matmul(dt_tiles[l][:], Sb[:, 0, :], A8[:, lr, :],
                                 start=(c == 0), stop=False,
                                 skip_group_check=True)
            for l in pairs:
                lr = 2 * (l - g * GRP)
                nc.tensor.matmul(dt_tiles[l][:], Sb[:, 1, :], A8[:, lr + 1, :],
                                 start=False, stop=(c == NCHUNK - 1),
                                 skip_group_check=True)

        def emit_group_tail(g):
            for l in range(g * GRP, (g + 1) * GRP):
                js = BLK * (2 * l // BLK)
                E = epool.tile([P, WIN], bf16, tag="E", name=f"E{l}")
                nc.scalar.activation(out=E[:], in_=dt_tiles[l][:], func=EXP,
                                     scale=-2.0, accum_out=Rt[:, l:l + 1])
                nc.gpsimd.tensor_add(
                    ACC[:, js + CA_LO:js + CA_HI],
                    ACC[:, js + CA_LO:js + CA_HI],
                    E[:, CA_LO:CA_HI],
                )

        # group 0 interleaved with the GEMM (2-chunk lookahead)
        emit_gemm_chunk(0)
        emit_gemm_chunk(1)
        dt_tiles = {l: psum_d.tile([P, WIN], f32, tag="D", name=f"D{l}")
                    for l in range(GRP)}
        for c in range(NCHUNK):
            if c + 2 < NCHUNK:
                emit_gemm_chunk(c + 2)
            emit_pairwise_chunk(0, c)
        emit_group_tail(0)

        for g in range(1, NPAIR // GRP):
            dt_tiles = {l: psum_d.tile([P, WIN], f32, tag="D", name=f"D{l}")
                        for l in range(g * GRP, (g + 1) * GRP)}
            for c in range(NCHUNK):
                emit_pairwise_chunk(g, c)
            emit_group_tail(g)

        nc.sync.dma_start(out=r_out[:], in_=Rt[:])
        nc.sync.dma_start(out=acc_out[:], in_=ACC[:])


def _program():
    if "nc" in _CACHE:
        return _CACHE["nc"]
    import concourse.bacc as bacc
    import concourse.tile as tile
    from concourse import mybir

    f32 = mybir.dt.float32
    nc = bacc.Bacc(
        "TRN2",
        target_bir_lowering=False,
        debug=False,
        num_devices=NCORES,
    )
    bf16 = mybir.dt.bfloat16
    x_in = nc.dram_tensor("x", [IN_F, B], bf16, kind="ExternalInput").ap()
    t_in = nc.dram_tensor("T3", [NCHUNK, NCC, P, P], bf16, kind="ExternalInput").ap()
    s_in = nc.dram_tensor("S", [P, 2, P], f32, kind="ExternalInput").ap()
    r_out = nc.dram_tensor("R", [P, NPAIR], f32, kind="ExternalOutput").ap()
    acc_out = nc.dram_tensor("ACC", [P, ACC_W], f32, kind="ExternalOutput").ap()

    with tile.TileContext(nc) as tc:
        _build_kernel(tc, r_out, acc_out, x_in, t_in, s_in)
    nc.compile()
    _CACHE["nc"] = nc
    return nc


def _in_maps(x, t3):
    import ml_dtypes

    bf = ml_dtypes.bfloat16
    s = _stationary()
    t3b = np.ascontiguousarray(t3.astype(bf))
    xb = x.astype(bf)
    maps = []
    for c in range(NCORES):
        xc = np.ascontiguousarray(np.roll(xb, -RPC * c, axis=0).T)  # [1024, 512]
        maps.append({"x": xc, "T3": t3b, "S": s})
    return maps


def _assemble(x, results):
    feats = np.zeros((B, OUT_F), np.float32)
    jl = np.arange(ACC_W)
    for c in range(NCORES):
        R = np.asarray(results[c]["R"], np.float32)        # [128, 32]
        ACCv = np.asarray(results[c]["ACC"], np.float32)   # [128, 320]
        base = RPC * c
        for l in range(NPAIR):
            feats[base + 2 * l] += R[:OUT_F, l]
            feats[base + 2 * l + 1] += R[OUT_F:, l]
        fold = (ACCv[:OUT_F] + ACCv[OUT_F:]).T             # [320, 64]
        gj = (jl + base) % B
        np.add.at(feats, gj, fold)
    return np.concatenate([x, feats], axis=1)


def _ensure_ntff_hook():
    """Register the axon NTFF profile hook (the image's antenv stub lacks
    axon_hooks, so concourse's trace=True path can't find it otherwise)."""
    import types

    if "antenv.axon_hooks" in sys.modules:
        return
    try:
        from trn_agent_boot.trn_boot import _ntff_profile_via_ctypes

        hook = _ntff_profile_via_ctypes("/opt/axon/libaxon_pjrt.so")
    except Exception:
        hook = None
    mod = types.ModuleType("antenv.axon_hooks")
    mod.get_axon_ntff_profile_hook = lambda: hook
    mod.set_axon_ntff_profile_hook = lambda h: None
    sys.modules["antenv.axon_hooks"] = mod


def _okc_major_t3(T):
    """T [1024, 64, 32] (or flat) -> 0.5-scaled k-major okc-major blocks
    [16, 8, 128, 128]: t3[okc, cc, p, j] = 0.5 * T2km[cc*128+p, okc*128+j]."""
    t = np.asarray(T, np.float32).reshape(IN_F, OUT_F, K)
    t2 = t.transpose(0, 2, 1).reshape(IN_F, OK) * 0.5   # k-major, pre-scaled
    t3 = t2.reshape(NCC, P, NCHUNK, P).transpose(2, 0, 1, 3)
    return np.ascontiguousarray(t3)


def run(x, T, trace=False):
    """Returns (output, BassKernelResults)."""
    if trace:
        _ensure_ntff_hook()
    from concourse.bass_utils import run_bass_kernel_spmd

    x = np.ascontiguousarray(np.asarray(x, np.float32))
    t3 = _okc_major_t3(T)
    nc = _program()
    res = run_bass_kernel_spmd(
        nc, _in_maps(x, t3), list(range(NCORES)), trace=trace
    )
    return _assemble(x, res.results), res


def kernel(x, T):
    out, _ = run(x, T, trace=False)
    return out


# revision 10
# speedup vs baseline: 1.0192x; 1.0192x over previous
"""
MiniBatchDiscrimination on 8 Trainium2 NeuronCores (Bass/Tile, SPMD).

Reference computation (jax):
    M = (x @ T.reshape(1024, 2048)).reshape(512, 64, 32)
    abs_diff[i, j, o] = sum_k |M[j, o, k] - M[i, o, k]|        # [512, 512, 64]
    feats[i, o]      = sum_j exp(-abs_diff[i, j, o])           # [512, 64]
    out = concat([x, feats], axis=1)                           # [512, 1088]

Distribution strategy (SPMD: one program on 8 cores; all per-core variation
rides in the input data): every core receives x^T ROLLED by -64*core rows
plus the full (replicated) T, computes the full M^T = (x @ T)^T locally, and
produces features for its LOCAL rows 0..63.

Symmetric halving via a cyclic block-window: with 64 blocks of BLK=8 rows,
the row-pass of row i covers columns [8*(i//8), +264) -- its own block plus
the next 32 blocks (no wrap occurs locally since local rows live in blocks
0..7).  For block-distance 1..31 pairs the transpose term is supplied by a
column-accumulator over the window's blocks +1..+31; block-distance-32 pairs
are computed by BOTH owning rows' passes (and excluded from the col-acc), so
every unordered pair contributes to both features exactly once.  This is
0.52x the full pairwise work.  The per-core roll keeps it SPMD-exact: the
scheme only references LOCAL block structure, and the host re-rolls the
column accumulator when folding.

M^T uses a K-MAJOR column order (flat index = k*64 + o) so every one of the
16 partition-chunks maps to output features with the SAME [128, 64] 0/1
stationary; row i0 of a pair reduces into PSUM partitions 0..63 and row i1
into 64..127 (PE tile positioning), sharing one PSUM tile.  T is PRE-SCALED
by 0.5 on the host (exact in bf16); the exp uses scale=-2 to compensate
(harmless numerically, keeps all dtypes comfortably in range).

Device pipeline per core:
  1. PE warm-up burst (~10 dummy matmuls) releases the HAM clock throttle
     before the real GEMM arrives; DMA x^T (1MB) interleaved with T's first
     output-chunk block, then the rest of T okc-major (4MB total).
  2. PE GEMM  M^T = T^T @ x^T (bf16 in, fp32 accum) per okc chunk, evicted
     by ScalarE to bf16 MT [128, 16, 512] plus an fp32 upcast MTf (bias/
     scalar operands must be fp32 AND bit-exact with the bf16 values so
     self-distances are exactly 0).  The first two chunks are emitted ahead,
     then each remaining chunk is emitted interleaved with group 0's
     pairwise work so every consumer waits only on its own chunk.
  3. Per group of GRP=4 row-pairs (one 8-row block), chunk-major:
       - |M^T - m_i| over the 264-wide window into a bf16 tile [128, 8, 264]:
         ScalarE activation(Abs, scale=-1, bias=m_i) for some rows, DVE
         tensor_scalar(subtract) + one batched u16-view bitwise-AND 0x7FFF
         (sign-bit clear = abs) for the rest.
       - k-reduction on PE: per chunk one matmul per row with the shared
         [128, 64] 0/1 stationary slab, accumulating D [128, 264] in PSUM.
       - ScalarE activation(Exp, scale=-2, accum_out) fuses exp(-2D) and
         the window row-sum -> R[:, l]; GpSimdE adds E's blocks +1..+31 into
         the column accumulator ACC [128, 320].
  4. DMA R [128, 32] and ACC [128, 320] back; host scatters/folds.

bf16 in the pairwise stage is safe here: pairwise L1 half-distances of this
input distribution are ~250-700 (exp underflows to exactly 0 in fp32, as in
the reference itself -- min off-diagonal distance measured 502), and
self-terms are exactly 0 in any precision.
"""

import os
import sys

import numpy as np

for _p in ("/opt/trn_rl_repo", "/root/.axon_site/_ro/trn_rl_repo"):
    if os.path.isdir(_p) and _p not in sys.path:
        sys.path.insert(0, _p)

B = 512          # batch
IN_F = 1024      # in_features
OUT_F = 64       # out_features
K = 32           # intermediate dim
OK = OUT_F * K   # 2048 flattened (k, o) -- k-major
P = 128          # partitions
NCHUNK = OK // P      # 16
NCC = IN_F // P       # 8 contraction chunks for the GEMM
NCORES = 8
RPC = B // NCORES     # rows per core = 64
NPAIR = RPC // 2      # 32 row-pairs per core
BLK = int(os.environ.get("MBD_BLK", "8"))   # window block size
WIN = BLK + 256       # own block + 256/BLK more blocks
CA_LO, CA_HI = BLK, 256  # window-relative col-acc range (blocks +1..+m-1)
ACC_W = 320           # max jstart (56) + WIN (264)

# abs-diff engine split: chunks in ACT_CHUNKS run fully on ScalarE, the
# first non-ACT chunk donates SPLIT_ROWS rows to ScalarE, rest on DVE
ACT_CHUNKS = tuple(
    int(c) for c in os.environ.get("MBD_ACT", "2,5,8,11,14").split(",") if c != ""
)
A_BUFS = int(os.environ.get("MBD_ABUFS", "20"))
GRP = int(os.environ.get("MBD_GRP", "4"))  # row-pairs per PSUM group
SPLIT_ROWS = int(os.environ.get("MBD_SPLIT", "6"))  # rows of one DVE chunk -> ACT
N_WARM = int(os.environ.get("MBD_WARM", "10"))  # PE warm-up matmuls

_CACHE = {}


def _stationary():
    """[128, 2, 128] 0/1 matrices: partition (k2, o64) -> PSUM row (k-major).
    Slab 0 maps to rows o (pair row i0), slab 1 to rows 64+o (row i1)."""
    s = np.zeros((P, 2, P), np.float32)
    for p in range(P):
        s[p, 0, p % OUT_F] = 1.0
        s[p, 1, OUT_F + p % OUT_F] = 1.0
    return s


def _build_kernel(tc, r_out, acc_out, x_in, t_in, s_in):
    import concourse.bass as bass
    from concourse import mybir

    nc = tc.nc
    f32 = mybir.dt.float32
    bf16 = mybir.dt.bfloat16
    u16 = mybir.dt.uint16
    SUB = mybir.AluOpType.subtract
    AND = mybir.AluOpType.bitwise_and
    ABS = mybir.ActivationFunctionType.Abs
    EXP = mybir.ActivationFunctionType.Exp

    from contextlib import ExitStack

    with ExitStack() as ctx:
        const = ctx.enter_context(tc.tile_pool(name="const", bufs=1))
        big = ctx.enter_context(tc.tile_pool(name="big", bufs=1))
        staging = ctx.enter_context(tc.tile_pool(name="staging", bufs=1))
        psum_g = ctx.enter_context(tc.tile_pool(name="psum_g", bufs=2, space="PSUM"))

        MT = big.tile([P, NCHUNK, B], bf16)             # 2MB
        MTf = big.tile([P, NCHUNK, B], f32)             # 4MB
        Sb = const.tile([P, 2, P], bf16)
        Rt = const.tile([P, NPAIR], f32)
        ACC = const.tile([P, ACC_W], f32)
        Wz = const.tile([P, B], bf16)
        nc.vector.memset(ACC[:], 0.0)
        nc.vector.memset(Wz[:], 0.0)

        # ---- PE warm-up: release the HAM clock throttle while DMAs fly ----
        pw = psum_g.tile([P, B], f32, tag="pg", name="warm")
        for _ in range(N_WARM):
            nc.tensor.matmul(pw[:], Wz[:, 0:P], Wz[:], start=True, stop=True,
                             skip_group_check=True)

        # ---- input DMAs: x^T interleaved with T's okc-0 block, rest of T
        #      okc-major so GEMM chunk okc unblocks in order ----
        XTb = staging.tile([P, NCC, B], bf16)           # 1MB
        Tb = staging.tile([P, NCHUNK, NCC, P], bf16)    # 4MB
        for cc in range(NCC):
            nc.sync.dma_start(out=XTb[:, cc, :], in_=x_in[cc * P:(cc + 1) * P, :])
            nc.sync.dma_start(out=Tb[:, 0, cc, :], in_=t_in[0, cc, :, :])
        for okc in range(1, NCHUNK):
            for cc in range(NCC):
                nc.sync.dma_start(out=Tb[:, okc, cc, :], in_=t_in[okc, cc, :, :])
        Sf = staging.tile([P, 2, P], f32)
        nc.sync.dma_start(out=Sf[:], in_=s_in[:])
        nc.vector.tensor_copy(out=Sb[:], in_=Sf[:])

        def emit_gemm_mms(okc):
            pg = psum_g.tile([P, B], f32, tag="pg", name=f"pg{okc}")
            for cc in range(NCC):
                nc.tensor.matmul(
                    pg[:],
                    Tb[:, okc, cc, :],
                    XTb[:, cc, :],
                    start=(cc == 0),
                    stop=(cc == NCC - 1),
                )
            return pg

        def emit_evict(okc, pg):
            nc.scalar.copy(out=MT[:, okc, :], in_=pg[:])
            nc.scalar.copy(out=MTf[:, okc, :], in_=MT[:, okc, :])

        # ---- pairwise stage ----
        apool = ctx.enter_context(tc.tile_pool(name="apool", bufs=A_BUFS))
        epool = ctx.enter_context(tc.tile_pool(name="epool", bufs=6))
        psum_d = ctx.enter_context(tc.tile_pool(name="psum_d", bufs=6, space="PSUM"))
        act_chunks = set(ACT_CHUNKS)
        split_chunk = next(c for c in range(NCHUNK) if c not in act_chunks)

        NR = 2 * GRP  # rows per group
        dt_tiles = {}

        def emit_pairwise_chunk(g, c):
            pairs = range(g * GRP, (g + 1) * GRP)
            r0 = 2 * g * GRP
            gjs = BLK * (r0 // BLK)
            A8 = apool.tile([P, NR, WIN], bf16, tag="A8", name=f"A8_{c}_{g}")
            n_act = (NR if c in act_chunks
                     else (SPLIT_ROWS if c == split_chunk else 0))
            nd = NR - n_act
            for r in range(NR):
                row = r0 + r
                if r >= nd:
                    nc.scalar.activation(
                        out=A8[:, r, :], in_=MT[:, c, gjs:gjs + WIN], func=ABS,
                        bias=MTf[:, c, row:row + 1], scale=-1.0,
                    )
                else:
                    nc.vector.tensor_scalar(
                        out=A8[:, r, :],
                        in0=MT[:, c, gjs:gjs + WIN],
                        scalar1=MTf[:, c, row:row + 1],
                        scalar2=None, op0=SUB,
                    )
            if nd:
                # batched sign-bit clear (= abs) over the DVE-written rows
                Au = A8[:, 0:nd, :].bitcast(u16)
                nc.vector.tensor_scalar(
                    out=Au, in0=Au, scalar1=0x7FFF, scalar2=None, op0=AND,
                )
            for l in pairs:
                lr = 2 * (l - g * GRP)
                nc.tensor.matmul(dt_tiles[l][:], Sb[:, 0, :], A8[:, lr, :],
                                 start=(c == 0), stop=False,
                                 skip_group_check=True)
            for l in pairs:
                lr = 2 * (l - g * GRP)
                nc.tensor.matmul(dt_tiles[l][:], Sb[:, 1, :], A8[:, lr + 1, :],
                                 start=False, stop=(c == NCHUNK - 1),
                                 skip_group_check=True)

        def emit_group_tail(g):
            for l in range(g * GRP, (g + 1) * GRP):
                js = BLK * (2 * l // BLK)
                E = epool.tile([P, WIN], bf16, tag="E", name=f"E{l}")
                nc.scalar.activation(out=E[:], in_=dt_tiles[l][:], func=EXP,
                                     scale=-2.0, accum_out=Rt[:, l:l + 1])
                nc.gpsimd.tensor_add(
                    ACC[:, js + CA_LO:js + CA_HI],
                    ACC[:, js + CA_LO:js + CA_HI],
                    E[:, CA_LO:CA_HI],
                )

        # All GEMM matmuls upfront so the PE FIFO runs them dense and
        # contention-free; evictions are emitted interleaved with group 0's
        # pairwise work so each consumer waits only on its own chunk.
        pgs = [emit_gemm_mms(okc) for okc in range(NCHUNK)]
        dt_tiles = {l: psum_d.tile([P, WIN], f32, tag="D", name=f"D{l}")
                    for l in range(GRP)}
        for c in range(NCHUNK):
            emit_evict(c, pgs[c])
            emit_pairwise_chunk(0, c)
        emit_group_tail(0)

        for g in range(1, NPAIR // GRP):
            dt_tiles = {l: psum_d.tile([P, WIN], f32, tag="D", name=f"D{l}")
                        for l in range(g * GRP, (g + 1) * GRP)}
            for c in range(NCHUNK):
                emit_pairwise_chunk(g, c)
            emit_group_tail(g)

        nc.sync.dma_start(out=r_out[:], in_=Rt[:])
        nc.sync.dma_start(out=acc_out[:], in_=ACC[:])


def _program():
    if "nc" in _CACHE:
        return _CACHE["nc"]
    import concourse.bacc as bacc
    import concourse.tile as tile
    from concourse import mybir

    f32 = mybir.dt.float32
    nc = bacc.Bacc(
        "TRN2",
        target_bir_lowering=False,
        debug=False,
        num_devices=NCORES,
    )
    bf16 = mybir.dt.bfloat16
    x_in = nc.dram_tensor("x", [IN_F, B], bf16, kind="ExternalInput").ap()
    t_in = nc.dram_tensor("T3", [NCHUNK, NCC, P, P], bf16, kind="ExternalInput").ap()
    s_in = nc.dram_tensor("S", [P, 2, P], f32, kind="ExternalInput").ap()
    r_out = nc.dram_tensor("R", [P, NPAIR], f32, kind="ExternalOutput").ap()
    acc_out = nc.dram_tensor("ACC", [P, ACC_W], f32, kind="ExternalOutput").ap()

    with tile.TileContext(nc) as tc:
        _build_kernel(tc, r_out, acc_out, x_in, t_in, s_in)
    nc.compile()
    _CACHE["nc"] = nc
    return nc


def _in_maps(x, t3):
    import ml_dtypes

    bf = ml_dtypes.bfloat16
    s = _stationary()
    t3b = np.ascontiguousarray(t3.astype(bf))
    xb = x.astype(bf)
    maps = []
    for c in range(NCORES):
        xc = np.ascontiguousarray(np.roll(xb, -RPC * c, axis=0).T)  # [1024, 512]
        maps.append({"x": xc, "T3": t3b, "S": s})
    return maps


def _assemble(x, results):
    feats = np.zeros((B, OUT_F), np.float32)
    jl = np.arange(ACC_W)
    for c in range(NCORES):
        R = np.asarray(results[c]["R"], np.float32)        # [128, 32]
        ACCv = np.asarray(results[c]["ACC"], np.float32)   # [128, 320]
        base = RPC * c
        for l in range(NPAIR):
            feats[base + 2 * l] += R[:OUT_F, l]
            feats[base + 2 * l + 1] += R[OUT_F:, l]
        fold = (ACCv[:OUT_F] + ACCv[OUT_F:]).T             # [320, 64]
        gj = (jl + base) % B
        np.add.at(feats, gj, fold)
    return np.concatenate([x, feats], axis=1)


def _ensure_ntff_hook():
    """Register the axon NTFF profile hook (the image's antenv stub lacks
    axon_hooks, so concourse's trace=True path can't find it otherwise)."""
    import types

    if "antenv.axon_hooks" in sys.modules:
        return
    try:
        from trn_agent_boot.trn_boot import _ntff_profile_via_ctypes

        hook = _ntff_profile_via_ctypes("/opt/axon/libaxon_pjrt.so")
    except Exception:
        hook = None
    mod = types.ModuleType("antenv.axon_hooks")
    mod.get_axon_ntff_profile_hook = lambda: hook
    mod.set_axon_ntff_profile_hook = lambda h: None
    sys.modules["antenv.axon_hooks"] = mod


def _okc_major_t3(T):
    """T [1024, 64, 32] (or flat) -> 0.5-scaled k-major okc-major blocks
    [16, 8, 128, 128]: t3[okc, cc, p, j] = 0.5 * T2km[cc*128+p, okc*128+j]."""
    t = np.asarray(T, np.float32).reshape(IN_F, OUT_F, K)
    t2 = t.transpose(0, 2, 1).reshape(IN_F, OK) * 0.5   # k-major, pre-scaled
    t3 = t2.reshape(NCC, P, NCHUNK, P).transpose(2, 0, 1, 3)
    return np.ascontiguousarray(t3)


def run(x, T, trace=False):
    """Returns (output, BassKernelResults)."""
    if trace:
        _ensure_ntff_hook()
    from concourse.bass_utils import run_bass_kernel_spmd

    x = np.ascontiguousarray(np.asarray(x, np.float32))
    t3 = _okc_major_t3(T)
    nc = _program()
    res = run_bass_kernel_spmd(
        nc, _in_maps(x, t3), list(range(NCORES)), trace=trace
    )
    return _assemble(x, res.results), res


def kernel(x, T):
    out, _ = run(x, T, trace=False)
    return out


# revision 13
# speedup vs baseline: 1.0246x; 1.0054x over previous
"""
MiniBatchDiscrimination on 8 Trainium2 NeuronCores (Bass/Tile, SPMD).

Reference computation (jax):
    M = (x @ T.reshape(1024, 2048)).reshape(512, 64, 32)
    abs_diff[i, j, o] = sum_k |M[j, o, k] - M[i, o, k]|        # [512, 512, 64]
    feats[i, o]      = sum_j exp(-abs_diff[i, j, o])           # [512, 64]
    out = concat([x, feats], axis=1)                           # [512, 1088]

Distribution strategy (SPMD: one program on 8 cores; all per-core variation
rides in the input data): every core receives x^T ROLLED by -64*core rows
plus the full (replicated) T, computes the full M^T = (x @ T)^T locally, and
produces features for its LOCAL rows 0..63.

Symmetric halving via a cyclic block-window: with 64 blocks of BLK=8 rows,
the row-pass of row i covers columns [8*(i//8), +264) -- its own block plus
the next 32 blocks (no wrap occurs locally since local rows live in blocks
0..7).  For block-distance 1..31 pairs the transpose term is supplied by a
column-accumulator over the window's blocks +1..+31; block-distance-32 pairs
are computed by BOTH owning rows' passes (and excluded from the col-acc), so
every unordered pair contributes to both features exactly once.  This is
0.52x the full pairwise work.  The per-core roll keeps it SPMD-exact: the
scheme only references LOCAL block structure, and the host re-rolls the
column accumulator when folding.

M^T uses a K-MAJOR column order (flat index = k*64 + o) so every one of the
16 partition-chunks maps to output features with the SAME [128, 64] 0/1
stationary; row i0 of a pair reduces into PSUM partitions 0..63 and row i1
into 64..127 (PE tile positioning), sharing one PSUM tile.  T is PRE-SCALED
by 0.5 on the host (exact in bf16); the exp uses scale=-2 to compensate
(harmless numerically, keeps all dtypes comfortably in range).

Device pipeline per core:
  1. PE warm-up burst (~10 dummy matmuls) releases the HAM clock throttle
     before the real GEMM arrives; DMA x^T (1MB) interleaved with T's first
     output-chunk block, then the rest of T okc-major (4MB total).
  2. PE GEMM  M^T = T^T @ x^T (bf16 in, fp32 accum) per okc chunk, evicted
     by ScalarE to bf16 MT [128, 16, 512] plus an fp32 upcast MTf (bias/
     scalar operands must be fp32 AND bit-exact with the bf16 values so
     self-distances are exactly 0).  The first two chunks are emitted ahead,
     then each remaining chunk is emitted interleaved with group 0's
     pairwise work so every consumer waits only on its own chunk.
  3. Per group of GRP=4 row-pairs (one 8-row block), chunk-major:
       - |M^T - m_i| over the 264-wide window into a bf16 tile [128, 8, 264]:
         ScalarE activation(Abs, scale=-1, bias=m_i) for some rows, DVE
         tensor_scalar(subtract) + one batched u16-view bitwise-AND 0x7FFF
         (sign-bit clear = abs) for the rest.
       - k-reduction on PE: per chunk one matmul per row with the shared
         [128, 64] 0/1 stationary slab, accumulating D [128, 264] in PSUM.
       - ScalarE activation(Exp, scale=-2, accum_out) fuses exp(-2D) and
         the window row-sum -> R[:, l]; GpSimdE adds E's blocks +1..+31 into
         the column accumulator ACC [128, 320].
  4. DMA R [128, 32] and ACC [128, 320] back; host scatters/folds.

bf16 in the pairwise stage is safe here: pairwise L1 half-distances of this
input distribution are ~250-700 (exp underflows to exactly 0 in fp32, as in
the reference itself -- min off-diagonal distance measured 502), and
self-terms are exactly 0 in any precision.
"""

import os
import sys

import numpy as np

for _p in ("/opt/trn_rl_repo", "/root/.axon_site/_ro/trn_rl_repo"):
    if os.path.isdir(_p) and _p not in sys.path:
        sys.path.insert(0, _p)

B = 512          # batch
IN_F = 1024      # in_features
OUT_F = 64       # out_features
K = 32           # intermediate dim
OK = OUT_F * K   # 2048 flattened (k, o) -- k-major
P = 128          # partitions
NCHUNK = OK // P      # 16
NCC = IN_F // P       # 8 contraction chunks for the GEMM
NCORES = 8
RPC = B // NCORES     # rows per core = 64
NPAIR = RPC // 2      # 32 row-pairs per core
BLK = int(os.environ.get("MBD_BLK", "8"))   # window block size
WIN = BLK + 256       # own block + 256/BLK more blocks
CA_LO, CA_HI = BLK, 256  # window-relative col-acc range (blocks +1..+m-1)
ACC_W = 320           # max jstart (56) + WIN (264)

# abs-diff engine split: chunks in ACT_CHUNKS run fully on ScalarE, the
# first non-ACT chunk donates SPLIT_ROWS rows to ScalarE, rest on DVE
ACT_CHUNKS = tuple(
    int(c) for c in os.environ.get("MBD_ACT", "2,5,8,11,14").split(",") if c != ""
)
A_BUFS = int(os.environ.get("MBD_ABUFS", "20"))
GRP = int(os.environ.get("MBD_GRP", "4"))  # row-pairs per PSUM group
SPLIT_ROWS = int(os.environ.get("MBD_SPLIT", "7"))  # rows of one DVE chunk -> ACT
N_WARM = int(os.environ.get("MBD_WARM", "10"))  # PE warm-up matmuls

_CACHE = {}


def _stationary():
    """[128, 2, 128] 0/1 matrices: partition (k2, o64) -> PSUM row (k-major).
    Slab 0 maps to rows o (pair row i0), slab 1 to rows 64+o (row i1)."""
    s = np.zeros((P, 2, P), np.float32)
    for p in range(P):
        s[p, 0, p % OUT_F] = 1.0
        s[p, 1, OUT_F + p % OUT_F] = 1.0
    return s


def _build_kernel(tc, r_out, acc_out, x_in, t_in, s_in):
    import concourse.bass as bass
    from concourse import mybir

    nc = tc.nc
    f32 = mybir.dt.float32
    bf16 = mybir.dt.bfloat16
    u16 = mybir.dt.uint16
    SUB = mybir.AluOpType.subtract
    AND = mybir.AluOpType.bitwise_and
    ABS = mybir.ActivationFunctionType.Abs
    EXP = mybir.ActivationFunctionType.Exp

    from contextlib import ExitStack

    with ExitStack() as ctx:
        const = ctx.enter_context(tc.tile_pool(name="const", bufs=1))
        big = ctx.enter_context(tc.tile_pool(name="big", bufs=1))
        staging = ctx.enter_context(tc.tile_pool(name="staging", bufs=1))
        psum_g = ctx.enter_context(tc.tile_pool(name="psum_g", bufs=2, space="PSUM"))

        MT = big.tile([P, NCHUNK, B], bf16)             # 2MB
        MTf = big.tile([P, NCHUNK, B], f32)             # 4MB
        Sb = const.tile([P, 2, P], bf16)
        Rt = const.tile([P, NPAIR], f32)
        ACC = const.tile([P, ACC_W], f32)
        Wz = const.tile([P, B], bf16)
        nc.vector.memset(ACC[:], 0.0)
        nc.vector.memset(Wz[:], 0.0)

        # ---- PE warm-up: release the HAM clock throttle while DMAs fly ----
        pw = psum_g.tile([P, B], f32, tag="pg", name="warm")
        for _ in range(N_WARM):
            nc.tensor.matmul(pw[:], Wz[:, 0:P], Wz[:], start=True, stop=True,
                             skip_group_check=True)

        # ---- input DMAs: x^T interleaved with T's okc-0 block, rest of T
        #      okc-major so GEMM chunk okc unblocks in order ----
        XTb = staging.tile([P, NCC, B], bf16)           # 1MB
        Tb = staging.tile([P, NCHUNK, NCC, P], bf16)    # 4MB
        for cc in range(NCC):
            nc.sync.dma_start(out=XTb[:, cc, :], in_=x_in[cc * P:(cc + 1) * P, :])
            nc.sync.dma_start(out=Tb[:, 0, cc, :], in_=t_in[0, cc, :, :])
        for okc in range(1, NCHUNK):
            for cc in range(NCC):
                nc.sync.dma_start(out=Tb[:, okc, cc, :], in_=t_in[okc, cc, :, :])
        Sf = staging.tile([P, 2, P], f32)
        nc.sync.dma_start(out=Sf[:], in_=s_in[:])
        nc.vector.tensor_copy(out=Sb[:], in_=Sf[:])

        def emit_gemm_mms(okc):
            pg = psum_g.tile([P, B], f32, tag="pg", name=f"pg{okc}")
            for cc in range(NCC):
                nc.tensor.matmul(
                    pg[:],
                    Tb[:, okc, cc, :],
                    XTb[:, cc, :],
                    start=(cc == 0),
                    stop=(cc == NCC - 1),
                )
            return pg

        def emit_evict(okc, pg):
            nc.scalar.copy(out=MT[:, okc, :], in_=pg[:])
            nc.scalar.copy(out=MTf[:, okc, :], in_=MT[:, okc, :])

        # ---- pairwise stage ----
        apool = ctx.enter_context(tc.tile_pool(name="apool", bufs=A_BUFS))
        epool = ctx.enter_context(tc.tile_pool(name="epool", bufs=6))
        psum_d = ctx.enter_context(tc.tile_pool(name="psum_d", bufs=6, space="PSUM"))
        act_chunks = set(ACT_CHUNKS)
        split_chunk = next(c for c in range(NCHUNK) if c not in act_chunks)
        # per-group chunk sweep order: pure-DVE chunks first, then the split
        # chunk, then ACT chunks -- each engine streams its own FIFO without
        # per-chunk cross-engine cadence (rings provide the slack)
        CHUNK_ORDER = ([c for c in range(NCHUNK)
                        if c not in act_chunks and c != split_chunk]
                       + [split_chunk] + sorted(act_chunks))

        NR = 2 * GRP  # rows per group
        dt_tiles = {}

        def emit_pairwise_chunk(g, c, first, last):
            pairs = range(g * GRP, (g + 1) * GRP)
            r0 = 2 * g * GRP
            gjs = BLK * (r0 // BLK)
            A8 = apool.tile([P, NR, WIN], bf16, tag="A8", name=f"A8_{c}_{g}")
            n_act = (NR if c in act_chunks
                     else (SPLIT_ROWS if c == split_chunk else 0))
            nd = NR - n_act
            for r in range(NR):
                row = r0 + r
                if r >= nd:
                    nc.scalar.activation(
                        out=A8[:, r, :], in_=MT[:, c, gjs:gjs + WIN], func=ABS,
                        bias=MTf[:, c, row:row + 1], scale=-1.0,
                    )
                else:
                    nc.vector.tensor_scalar(
                        out=A8[:, r, :],
                        in0=MT[:, c, gjs:gjs + WIN],
                        scalar1=MTf[:, c, row:row + 1],
                        scalar2=None, op0=SUB,
                    )
            if nd:
                # batched sign-bit clear (= abs) over the DVE-written rows
                Au = A8[:, 0:nd, :].bitcast(u16)
                nc.vector.tensor_scalar(
                    out=Au, in0=Au, scalar1=0x7FFF, scalar2=None, op0=AND,
                )
            for l in pairs:
                lr = 2 * (l - g * GRP)
                nc.tensor.matmul(dt_tiles[l][:], Sb[:, 0, :], A8[:, lr, :],
                                 start=first, stop=False,
                                 skip_group_check=True)
            for l in pairs:
                lr = 2 * (l - g * GRP)
                nc.tensor.matmul(dt_tiles[l][:], Sb[:, 1, :], A8[:, lr + 1, :],
                                 start=False, stop=last,
                                 skip_group_check=True)

        def emit_group_tail(g):
            for l in range(g * GRP, (g + 1) * GRP):
                js = BLK * (2 * l // BLK)
                E = epool.tile([P, WIN], bf16, tag="E", name=f"E{l}")
                nc.scalar.activation(out=E[:], in_=dt_tiles[l][:], func=EXP,
                                     scale=-2.0, accum_out=Rt[:, l:l + 1])
                nc.gpsimd.tensor_add(
                    ACC[:, js + CA_LO:js + CA_HI],
                    ACC[:, js + CA_LO:js + CA_HI],
                    E[:, CA_LO:CA_HI],
                )

        # GEMM matmuls dense and first in the PE FIFO (contention-free, HAM
        # stays warm), evictions immediately trailing on Scalar; the pairwise
        # groups then stream with DVE chunks leading each sweep.
        pgs = [emit_gemm_mms(okc) for okc in range(NCHUNK)]
        for okc in range(NCHUNK):
            emit_evict(okc, pgs[okc])

        for g in range(NPAIR // GRP):
            dt_tiles = {l: psum_d.tile([P, WIN], f32, tag="D", name=f"D{l}")
                        for l in range(g * GRP, (g + 1) * GRP)}
            for i, c in enumerate(CHUNK_ORDER):
                emit_pairwise_chunk(g, c, first=(i == 0), last=(i == NCHUNK - 1))
            emit_group_tail(g)

        nc.sync.dma_start(out=r_out[:], in_=Rt[:])
        nc.sync.dma_start(out=acc_out[:], in_=ACC[:])


def _program():
    if "nc" in _CACHE:
        return _CACHE["nc"]
    import concourse.bacc as bacc
    import concourse.tile as tile
    from concourse import mybir

    f32 = mybir.dt.float32
    nc = bacc.Bacc(
        "TRN2",
        target_bir_lowering=False,
        debug=False,
        num_devices=NCORES,
    )
    bf16 = mybir.dt.bfloat16
    x_in = nc.dram_tensor("x", [IN_F, B], bf16, kind="ExternalInput").ap()
    t_in = nc.dram_tensor("T3", [NCHUNK, NCC, P, P], bf16, kind="ExternalInput").ap()
    s_in = nc.dram_tensor("S", [P, 2, P], f32, kind="ExternalInput").ap()
    r_out = nc.dram_tensor("R", [P, NPAIR], f32, kind="ExternalOutput").ap()
    acc_out = nc.dram_tensor("ACC", [P, ACC_W], f32, kind="ExternalOutput").ap()

    with tile.TileContext(nc) as tc:
        _build_kernel(tc, r_out, acc_out, x_in, t_in, s_in)
    nc.compile()
    _CACHE["nc"] = nc
    return nc


def _in_maps(x, t3):
    import ml_dtypes

    bf = ml_dtypes.bfloat16
    s = _stationary()
    t3b = np.ascontiguousarray(t3.astype(bf))
    xb = x.astype(bf)
    maps = []
    for c in range(NCORES):
        xc = np.ascontiguousarray(np.roll(xb, -RPC * c, axis=0).T)  # [1024, 512]
        maps.append({"x": xc, "T3": t3b, "S": s})
    return maps


def _assemble(x, results):
    feats = np.zeros((B, OUT_F), np.float32)
    jl = np.arange(ACC_W)
    for c in range(NCORES):
        R = np.asarray(results[c]["R"], np.float32)        # [128, 32]
        ACCv = np.asarray(results[c]["ACC"], np.float32)   # [128, 320]
        base = RPC * c
        for l in range(NPAIR):
            feats[base + 2 * l] += R[:OUT_F, l]
            feats[base + 2 * l + 1] += R[OUT_F:, l]
        fold = (ACCv[:OUT_F] + ACCv[OUT_F:]).T             # [320, 64]
        gj = (jl + base) % B
        np.add.at(feats, gj, fold)
    return np.concatenate([x, feats], axis=1)


def _ensure_ntff_hook():
    """Register the axon NTFF profile hook (the image's antenv stub lacks
    axon_hooks, so concourse's trace=True path can't find it otherwise)."""
    import types

    if "antenv.axon_hooks" in sys.modules:
        return
    try:
        from trn_agent_boot.trn_boot import _ntff_profile_via_ctypes

        hook = _ntff_profile_via_ctypes("/opt/axon/libaxon_pjrt.so")
    except Exception:
        hook = None
    mod = types.ModuleType("antenv.axon_hooks")
    mod.get_axon_ntff_profile_hook = lambda: hook
    mod.set_axon_ntff_profile_hook = lambda h: None
    sys.modules["antenv.axon_hooks"] = mod


def _okc_major_t3(T):
    """T [1024, 64, 32] (or flat) -> 0.5-scaled k-major okc-major blocks
    [16, 8, 128, 128]: t3[okc, cc, p, j] = 0.5 * T2km[cc*128+p, okc*128+j]."""
    t = np.asarray(T, np.float32).reshape(IN_F, OUT_F, K)
    t2 = t.transpose(0, 2, 1).reshape(IN_F, OK) * 0.5   # k-major, pre-scaled
    t3 = t2.reshape(NCC, P, NCHUNK, P).transpose(2, 0, 1, 3)
    return np.ascontiguousarray(t3)


def run(x, T, trace=False):
    """Returns (output, BassKernelResults)."""
    if trace:
        _ensure_ntff_hook()
    from concourse.bass_utils import run_bass_kernel_spmd

    x = np.ascontiguousarray(np.asarray(x, np.float32))
    t3 = _okc_major_t3(T)
    nc = _program()
    res = run_bass_kernel_spmd(
        nc, _in_maps(x, t3), list(range(NCORES)), trace=trace
    )
    return _assemble(x, res.results), res


def kernel(x, T):
    out, _ = run(x, T, trace=False)
    return out


# revision 16
# speedup vs baseline: 1.2474x; 1.2174x over previous
"""
MiniBatchDiscrimination on 8 Trainium2 NeuronCores (Bass/Tile, SPMD).

Reference computation (jax):
    M = (x @ T.reshape(1024, 2048)).reshape(512, 64, 32)
    abs_diff[i, j, o] = sum_k |M[j, o, k] - M[i, o, k]|        # [512, 512, 64]
    feats[i, o]      = sum_j exp(-abs_diff[i, j, o])           # [512, 64]
    out = concat([x, feats], axis=1)                           # [512, 1088]

Distribution strategy (SPMD: one program on 8 cores; all per-core variation
rides in the input data): every core receives x^T ROLLED by -64*core rows
plus the full (replicated) T, computes the full M^T = (x @ T)^T locally, and
produces features for its LOCAL rows 0..63.

Symmetric halving via a cyclic block-window: with 64 blocks of BLK=8 rows,
the row-pass of row i covers columns [8*(i//8), +264) -- its own block plus
the next 32 blocks (no wrap occurs locally since local rows live in blocks
0..7).  For block-distance 1..31 pairs the transpose term is supplied by a
column-accumulator over the window's blocks +1..+31; block-distance-32 pairs
are computed by BOTH owning rows' passes (and excluded from the col-acc), so
every unordered pair contributes to both features exactly once.  This is
0.52x the full pairwise work.  The per-core roll keeps it SPMD-exact: the
scheme only references LOCAL block structure, and the host re-rolls the
column accumulator when folding.

M^T uses a K-MAJOR column order (flat index = k*64 + o) so every one of the
16 partition-chunks maps to output features with the SAME [128, 64] 0/1
stationary; row i0 of a pair reduces into PSUM partitions 0..63 and row i1
into 64..127 (PE tile positioning), sharing one PSUM tile.  T is PRE-SCALED
by 0.5 on the host (exact in bf16); the exp uses scale=-2 to compensate
(harmless numerically, keeps all dtypes comfortably in range).

Device pipeline per core:
  1. PE warm-up burst (~10 dummy matmuls) releases the HAM clock throttle
     before the real GEMM arrives; DMA x^T (1MB) interleaved with T's first
     output-chunk block, then the rest of T okc-major (4MB total).
  2. PE GEMM  M^T = T^T @ x^T (bf16 in, fp32 accum) per okc chunk, evicted
     by ScalarE to bf16 MT [128, 16, 512] plus an fp32 upcast MTf (bias/
     scalar operands must be fp32 AND bit-exact with the bf16 values so
     self-distances are exactly 0).  The first two chunks are emitted ahead,
     then each remaining chunk is emitted interleaved with group 0's
     pairwise work so every consumer waits only on its own chunk.
  3. Per group of GRP=4 row-pairs (one 8-row block), chunk-major:
       - |M^T - m_i| over the 264-wide window into a bf16 tile [128, 8, 264]:
         ScalarE activation(Abs, scale=-1, bias=m_i) for some rows, DVE
         tensor_scalar(subtract) + one batched u16-view bitwise-AND 0x7FFF
         (sign-bit clear = abs) for the rest.
       - k-reduction on PE: per chunk one matmul per row with the shared
         [128, 64] 0/1 stationary slab, accumulating D [128, 264] in PSUM.
       - ScalarE activation(Exp, scale=-2, accum_out) fuses exp(-2D) and
         the window row-sum -> R[:, l]; GpSimdE adds E's blocks +1..+31 into
         the column accumulator ACC [128, 320].
  4. DMA R [128, 32] and ACC [128, 320] back; host scatters/folds.

bf16 in the pairwise stage is safe here: pairwise L1 half-distances of this
input distribution are ~250-700 (exp underflows to exactly 0 in fp32, as in
the reference itself -- min off-diagonal distance measured 502), and
self-terms are exactly 0 in any precision.
"""

import os
import sys

import numpy as np

for _p in ("/opt/trn_rl_repo", "/root/.axon_site/_ro/trn_rl_repo"):
    if os.path.isdir(_p) and _p not in sys.path:
        sys.path.insert(0, _p)

B = 512          # batch
IN_F = 1024      # in_features
OUT_F = 64       # out_features
K = 32           # intermediate dim
OK = OUT_F * K   # 2048 flattened (k, o) -- k-major
P = 128          # partitions
NCHUNK = OK // P      # 16
NCC = IN_F // P       # 8 contraction chunks for the GEMM
NCORES = 8
RPC = B // NCORES     # rows per core = 64
NPAIR = RPC // 2      # 32 row-pairs per core
BLK = int(os.environ.get("MBD_BLK", "8"))   # window block size
WIN = BLK + 256       # own block + 256/BLK more blocks
CA_LO, CA_HI = BLK, 256  # window-relative col-acc range (blocks +1..+m-1)
ACC_W = 320           # max jstart (56) + WIN (264)

# abs-diff engine split: chunks in ACT_CHUNKS run fully on ScalarE, the
# first non-ACT chunk donates SPLIT_ROWS rows to ScalarE, rest on DVE
ACT_CHUNKS = tuple(
    int(c) for c in os.environ.get("MBD_ACT", "2,5,8,11,14").split(",") if c != ""
)
A_BUFS = int(os.environ.get("MBD_ABUFS", "20"))
GRP = int(os.environ.get("MBD_GRP", "4"))  # row-pairs per PSUM group
SPLIT_ROWS = int(os.environ.get("MBD_SPLIT", "7"))  # rows of one DVE chunk -> ACT
N_WARM = int(os.environ.get("MBD_WARM", "10"))  # PE warm-up matmuls

_CACHE = {}


def _stationary():
    """[128, 2, 128] 0/1 matrices: partition (k2, o64) -> PSUM row (k-major).
    Slab 0 maps to rows o (pair row i0), slab 1 to rows 64+o (row i1)."""
    s = np.zeros((P, 2, P), np.float32)
    for p in range(P):
        s[p, 0, p % OUT_F] = 1.0
        s[p, 1, OUT_F + p % OUT_F] = 1.0
    return s


def _build_kernel(tc, r_out, acc_out, x_in, t_in, s_in):
    import concourse.bass as bass
    from concourse import mybir

    nc = tc.nc
    f32 = mybir.dt.float32
    bf16 = mybir.dt.bfloat16
    u16 = mybir.dt.uint16
    SUB = mybir.AluOpType.subtract
    AND = mybir.AluOpType.bitwise_and
    ABS = mybir.ActivationFunctionType.Abs
    EXP = mybir.ActivationFunctionType.Exp

    from contextlib import ExitStack

    with ExitStack() as ctx:
        const = ctx.enter_context(tc.tile_pool(name="const", bufs=1))
        big = ctx.enter_context(tc.tile_pool(name="big", bufs=1))
        staging = ctx.enter_context(tc.tile_pool(name="staging", bufs=1))
        psum_g = ctx.enter_context(tc.tile_pool(name="psum_g", bufs=2, space="PSUM"))

        MT = big.tile([P, NCHUNK, B], bf16)             # 2MB
        MTf = big.tile([P, NCHUNK, B], f32)             # 4MB
        Sb = const.tile([P, 2, P], bf16)
        Rt = const.tile([P, NPAIR], f32)
        ACC = const.tile([P, ACC_W], f32)
        Wz = const.tile([P, B], bf16)
        nc.vector.memset(ACC[:], 0.0)
        nc.vector.memset(Wz[:], 0.0)

        # ---- PE warm-up: release the HAM clock throttle while DMAs fly ----
        pw = psum_g.tile([P, B], f32, tag="pg", name="warm")
        for _ in range(N_WARM):
            nc.tensor.matmul(pw[:], Wz[:, 0:P], Wz[:], start=True, stop=True,
                             skip_group_check=True)

        # ---- input DMAs: x^T interleaved with T's okc-0 block, rest of T
        #      okc-major so GEMM chunk okc unblocks in order ----
        XTb = staging.tile([P, NCC, B], bf16)           # 1MB
        Tb = staging.tile([P, NCHUNK, NCC, P], bf16)    # 4MB
        for cc in range(NCC):
            nc.sync.dma_start(out=XTb[:, cc, :], in_=x_in[cc * P:(cc + 1) * P, :])
            nc.sync.dma_start(out=Tb[:, cc, :, :], in_=t_in[cc, :, :])
        for okc in range(NCC, NCHUNK):
            nc.sync.dma_start(out=Tb[:, okc, :, :], in_=t_in[okc, :, :])
        Sf = staging.tile([P, 2, P], f32)
        nc.sync.dma_start(out=Sf[:], in_=s_in[:])
        nc.vector.tensor_copy(out=Sb[:], in_=Sf[:])

        def emit_gemm_mms(okc):
            pg = psum_g.tile([P, B], f32, tag="pg", name=f"pg{okc}")
            for cc in range(NCC):
                nc.tensor.matmul(
                    pg[:],
                    Tb[:, okc, cc, :],
                    XTb[:, cc, :],
                    start=(cc == 0),
                    stop=(cc == NCC - 1),
                )
            return pg

        def emit_evict(okc, pg):
            nc.scalar.copy(out=MT[:, okc, :], in_=pg[:])
            nc.scalar.copy(out=MTf[:, okc, :], in_=MT[:, okc, :])

        # ---- pairwise stage ----
        apool = ctx.enter_context(tc.tile_pool(name="apool", bufs=A_BUFS))
        epool = ctx.enter_context(tc.tile_pool(name="epool", bufs=6))
        psum_d = ctx.enter_context(tc.tile_pool(name="psum_d", bufs=6, space="PSUM"))
        act_chunks = set(ACT_CHUNKS)
        split_chunk = next(c for c in range(NCHUNK) if c not in act_chunks)
        # per-group chunk sweep order: pure-DVE chunks first, then the split
        # chunk, then ACT chunks -- each engine streams its own FIFO without
        # per-chunk cross-engine cadence (rings provide the slack)
        CHUNK_ORDER = ([c for c in range(NCHUNK)
                        if c not in act_chunks and c != split_chunk]
                       + [split_chunk] + sorted(act_chunks))

        NR = 2 * GRP  # rows per group
        dt_tiles = {}

        def emit_pairwise_chunk(g, c, first, last):
            pairs = range(g * GRP, (g + 1) * GRP)
            r0 = 2 * g * GRP
            gjs = BLK * (r0 // BLK)
            A8 = apool.tile([P, NR, WIN], bf16, tag="A8", name=f"A8_{c}_{g}")
            n_act = (NR if c in act_chunks
                     else (SPLIT_ROWS if c == split_chunk else 0))
            nd = NR - n_act
            for r in range(NR):
                row = r0 + r
                if r >= nd:
                    nc.scalar.activation(
                        out=A8[:, r, :], in_=MT[:, c, gjs:gjs + WIN], func=ABS,
                        bias=MTf[:, c, row:row + 1], scale=-1.0,
                    )
                else:
                    nc.vector.tensor_scalar(
                        out=A8[:, r, :],
                        in0=MT[:, c, gjs:gjs + WIN],
                        scalar1=MTf[:, c, row:row + 1],
                        scalar2=None, op0=SUB,
                    )
            if nd:
                # batched sign-bit clear (= abs) over the DVE-written rows
                Au = A8[:, 0:nd, :].bitcast(u16)
                nc.vector.tensor_scalar(
                    out=Au, in0=Au, scalar1=0x7FFF, scalar2=None, op0=AND,
                )
            for l in pairs:
                lr = 2 * (l - g * GRP)
                nc.tensor.matmul(dt_tiles[l][:], Sb[:, 0, :], A8[:, lr, :],
                                 start=first, stop=False,
                                 skip_group_check=True)
            for l in pairs:
                lr = 2 * (l - g * GRP)
                nc.tensor.matmul(dt_tiles[l][:], Sb[:, 1, :], A8[:, lr + 1, :],
                                 start=False, stop=last,
                                 skip_group_check=True)

        def emit_group_tail(g):
            for l in range(g * GRP, (g + 1) * GRP):
                js = BLK * (2 * l // BLK)
                E = epool.tile([P, WIN], bf16, tag="E", name=f"E{l}")
                nc.scalar.activation(out=E[:], in_=dt_tiles[l][:], func=EXP,
                                     scale=-2.0, accum_out=Rt[:, l:l + 1])
                nc.gpsimd.tensor_add(
                    ACC[:, js + CA_LO:js + CA_HI],
                    ACC[:, js + CA_LO:js + CA_HI],
                    E[:, CA_LO:CA_HI],
                )

        # GEMM matmuls dense and first in the PE FIFO (contention-free, HAM
        # stays warm), evictions immediately trailing on Scalar; the pairwise
        # groups then stream with DVE chunks leading each sweep.
        pgs = [emit_gemm_mms(okc) for okc in range(NCHUNK)]
        for okc in range(NCHUNK):
            emit_evict(okc, pgs[okc])

        for g in range(NPAIR // GRP):
            dt_tiles = {l: psum_d.tile([P, WIN], f32, tag="D", name=f"D{l}")
                        for l in range(g * GRP, (g + 1) * GRP)}
            for i, c in enumerate(CHUNK_ORDER):
                emit_pairwise_chunk(g, c, first=(i == 0), last=(i == NCHUNK - 1))
            emit_group_tail(g)

        nc.sync.dma_start(out=r_out[:], in_=Rt[:])
        nc.sync.dma_start(out=acc_out[:], in_=ACC[:])


def _program():
    if "nc" in _CACHE:
        return _CACHE["nc"]
    import concourse.bacc as bacc
    import concourse.tile as tile
    from concourse import mybir

    f32 = mybir.dt.float32
    nc = bacc.Bacc(
        "TRN2",
        target_bir_lowering=False,
        debug=False,
        num_devices=NCORES,
    )
    bf16 = mybir.dt.bfloat16
    x_in = nc.dram_tensor("x", [IN_F, B], bf16, kind="ExternalInput").ap()
    t_in = nc.dram_tensor("T3", [NCHUNK, P, NCC * P], bf16, kind="ExternalInput").ap()
    s_in = nc.dram_tensor("S", [P, 2, P], f32, kind="ExternalInput").ap()
    r_out = nc.dram_tensor("R", [P, NPAIR], f32, kind="ExternalOutput").ap()
    acc_out = nc.dram_tensor("ACC", [P, ACC_W], f32, kind="ExternalOutput").ap()

    with tile.TileContext(nc) as tc:
        _build_kernel(tc, r_out, acc_out, x_in, t_in, s_in)
    nc.compile()
    _CACHE["nc"] = nc
    return nc


def _in_maps(x, t3):
    import ml_dtypes

    bf = ml_dtypes.bfloat16
    s = _stationary()
    t3b = np.ascontiguousarray(t3.astype(bf))
    xb = x.astype(bf)
    maps = []
    for c in range(NCORES):
        xc = np.ascontiguousarray(np.roll(xb, -RPC * c, axis=0).T)  # [1024, 512]
        maps.append({"x": xc, "T3": t3b, "S": s})
    return maps


def _assemble(x, results):
    feats = np.zeros((B, OUT_F), np.float32)
    jl = np.arange(ACC_W)
    for c in range(NCORES):
        R = np.asarray(results[c]["R"], np.float32)        # [128, 32]
        ACCv = np.asarray(results[c]["ACC"], np.float32)   # [128, 320]
        base = RPC * c
        for l in range(NPAIR):
            feats[base + 2 * l] += R[:OUT_F, l]
            feats[base + 2 * l + 1] += R[OUT_F:, l]
        fold = (ACCv[:OUT_F] + ACCv[OUT_F:]).T             # [320, 64]
        gj = (jl + base) % B
        np.add.at(feats, gj, fold)
    return np.concatenate([x, feats], axis=1)


def _ensure_ntff_hook():
    """Register the axon NTFF profile hook (the image's antenv stub lacks
    axon_hooks, so concourse's trace=True path can't find it otherwise)."""
    import types

    if "antenv.axon_hooks" in sys.modules:
        return
    try:
        from trn_agent_boot.trn_boot import _ntff_profile_via_ctypes

        hook = _ntff_profile_via_ctypes("/opt/axon/libaxon_pjrt.so")
    except Exception:
        hook = None
    mod = types.ModuleType("antenv.axon_hooks")
    mod.get_axon_ntff_profile_hook = lambda: hook
    mod.set_axon_ntff_profile_hook = lambda h: None
    sys.modules["antenv.axon_hooks"] = mod


def _okc_major_t3(T):
    """T [1024, 64, 32] (or flat) -> 0.5-scaled k-major okc-major blocks
    [16, 128, 1024]: t3[okc, p, cc*128+j] = 0.5 * T2km[cc*128+p, okc*128+j]
    (one coarse DMA per okc with 2KB partition lines)."""
    t = np.asarray(T, np.float32).reshape(IN_F, OUT_F, K)
    t2 = t.transpose(0, 2, 1).reshape(IN_F, OK) * 0.5   # k-major, pre-scaled
    t3 = (t2.reshape(NCC, P, NCHUNK, P).transpose(2, 1, 0, 3)
          .reshape(NCHUNK, P, NCC * P))
    return np.ascontiguousarray(t3)


def run(x, T, trace=False):
    """Returns (output, BassKernelResults)."""
    if trace:
        _ensure_ntff_hook()
    from concourse.bass_utils import run_bass_kernel_spmd

    x = np.ascontiguousarray(np.asarray(x, np.float32))
    t3 = _okc_major_t3(T)
    nc = _program()
    res = run_bass_kernel_spmd(
        nc, _in_maps(x, t3), list(range(NCORES)), trace=trace
    )
    return _assemble(x, res.results), res


def kernel(x, T):
    out, _ = run(x, T, trace=False)
    return out
